# revision 7
# baseline (speedup 1.0000x reference)
"""Masked attention-weights kernel for Trainium2, 8-core data-parallel.

Computes, per batch b:
    q = relu(query @ Wq.T + bq)          [B, LQ, HID]
    k = relu(key   @ Wk.T + bk)          [B, LK, HID]
    logits = q @ k.T                     [B, LQ, LK]
    w = softmax(where(key_mask, logits, -1e9), axis=-1) * query_mask[:, :, None]

Strategy (fast path):
  * Data-parallel over batch B=32 across 8 NeuronCores, 4 batches ("slots")
    per core.  Batches are reassigned to (core, slot) so that per-slot
    packed sizes are minimized jointly.
  * Mask packing: only unmasked queries/keys are shipped, padded per slot
    to NQ[s] / NK[s] (multiples of 8).  Padded key columns have zero
    activations (relu(0*W + bk) with bk == 0), so after the row-max
    subtraction their softmax weight is exp(-max) ~ e^-150 -> flushes to
    0; they are additionally discarded by the host scatter, as are padded
    query rows.  No -1e9 bias tensor is needed at all (guarded: falls
    back to the dense kernel if bk != 0 or a key row is fully masked).
  * All matmul operands in fp16 (full PE rate, half the DMA/SBUF of
    fp32r; fp32 PSUM accumulation keeps the contraction exact).
    Measured end-to-end L2 error vs the fp32 reference: ~2.4e-3.
  * Softmax straight out of PSUM: reduce_max (vector) -> exp+accum
    (scalar, bias=-max) -> reciprocal + scale (vector) -> fp16 DMA out
    on the gpsimd queue (so output never queues behind input DMAs).
"""

import numpy as np

import concourse.bass as bass
import concourse.tile as tile
from concourse import mybir
from concourse.bass_utils import run_bass_kernel_spmd

N_CORES = 8
B, L, HID, D = 32, 1024, 1024, 1024
B_LOC = B // N_CORES
P = 128
DT = D // P
HT = HID // P
NEG = -1e9

F32 = mybir.dt.float32
F32R = mybir.dt.float32r
F16 = mybir.dt.float16

# test.py hooks: set TRACE_TMPDIR to profile; LAST_RESULT carries exec_time_ns
TRACE_TMPDIR = None
LAST_RESULT = None


def split_multiwaits(nc):
    """The walrus build in this container supports a single sync-wait per
    instruction; Tile's tail drain (and some scheduled insts) can carry
    several.  Split the extras into wait-only NOPs on the same engine,
    inserted immediately before the original instruction."""
    n_new = 0
    for fn in nc.m.functions:
        for blk in fn.blocks:
            new_insts = []
            for inst in blk.instructions:
                si = inst.sync_info
                if si is not None and si.on_wait is not None and len(si.on_wait) > 1:
                    waits = list(si.on_wait)
                    for w in waits[:-1]:
                        nop = mybir.InstNoOp(
                            name=f"{inst.name}-ws{n_new}", ins=[], outs=[]
                        )
                        nop.engine = inst.engine
                        nop.sync_info = mybir.SyncInfo(on_wait=[w], on_update=[])
                        new_insts.append(nop)
                        n_new += 1
                    si.on_wait = [waits[-1]]
                new_insts.append(inst)
            blk.instructions = new_insts
    return n_new


def _chunks(n):
    """PSUM free-dim chunking: one matmul if it fits a bank, else two equal
    halves (>=256 each keeps full PE rate)."""
    if n <= 512:
        return [(0, n)]
    h = n // 2
    return [(0, h), (h, n - h)]


def _relu(nc, dst2d, ps, chunks, bias_ap):
    if len(chunks) == 1:
        nc.scalar.activation(
            out=dst2d,
            in_=ps[:, 0, 0 : chunks[0][1]],
            func=mybir.ActivationFunctionType.Relu,
            bias=bias_ap,
            scale=1.0,
        )
    else:
        cw = chunks[0][1]
        nc.scalar.activation(
            out=dst2d.rearrange("p (a b) -> p a b", a=2),
            in_=ps[:, :, 0:cw],
            func=mybir.ActivationFunctionType.Relu,
            bias=bias_ap,
            scale=1.0,
        )


def build_bass_slotted(NQs, NKs, split=True):
    """Per-slot packed attention program.  Slot s processes one batch per
    core with packed query width NQs[s] and key width NKs[s]."""
    S = len(NQs)
    NQmax, NKmax = max(NQs), max(NKs)
    nc = bass.Bass()
    wq_p = nc.declare_dram_parameter("WqT", [D, HID], F16, isOutput=False)
    wk_p = nc.declare_dram_parameter("WkT", [D, HID], F16, isOutput=False)
    bq_p = nc.declare_dram_parameter("bq", [HID], F32, isOutput=False)
    bk_p = nc.declare_dram_parameter("bk", [HID], F32, isOutput=False)
    q_ps = [
        nc.declare_dram_parameter(f"qT{s}", [D, NQs[s]], F16, isOutput=False)
        for s in range(S)
    ]
    k_ps = [
        nc.declare_dram_parameter(f"kT{s}", [D, NKs[s]], F16, isOutput=False)
        for s in range(S)
    ]
    out_ps = [
        nc.declare_dram_parameter(f"out{s}", [NQs[s], NKs[s]], F16, isOutput=True)
        for s in range(S)
    ]

    with tile.TileContext(nc) as tc:
        with (
            tc.tile_pool(name="wsb", bufs=1) as w_pool,
            tc.tile_pool(name="const", bufs=1) as const_pool,
            tc.tile_pool(name="inp", bufs=1) as in_pool,
            tc.tile_pool(name="act", bufs=1) as act_pool,
            tc.tile_pool(name="wout", bufs=3) as wout_pool,
            tc.tile_pool(name="stat", bufs=9) as stat_pool,
            tc.tile_pool(name="ps", bufs=4, space="PSUM") as ps_pool,
        ):
            # ---- DMA plan.  Weight slices issue on the gpsimd queue in
            # parallel with slot-0 input slices on the sync queue (the
            # ~650ns/DMA descriptor-generation rate is what paces startup,
            # not bandwidth); biases ride the scalar queue.  Output DMAs
            # also go on the gpsimd queue, so they never wait behind input
            # prefetch on sync.
            bq_sb = const_pool.tile([P, HT], F32, tag="bq")
            nc.scalar.dma_start(out=bq_sb, in_=bq_p.ap().rearrange("(t p) -> p t", p=P))
            bk_sb = const_pool.tile([P, HT], F32, tag="bk")
            nc.scalar.dma_start(out=bk_sb, in_=bk_p.ap().rearrange("(t p) -> p t", p=P))

            wq_tiles = [
                w_pool.tile([P, HID], F16, tag=f"wq{i}", name=f"wq{i}")
                for i in range(DT)
            ]
            wk_tiles = [
                w_pool.tile([P, HID], F16, tag=f"wk{i}", name=f"wk{i}")
                for i in range(DT)
            ]
            q0_tiles = [
                in_pool.tile([P, NQs[0]], F16, tag=f"q0i{i}", name=f"q0i{i}")
                for i in range(DT)
            ]
            k0_tiles = [
                in_pool.tile([P, NKs[0]], F16, tag=f"k0i{i}", name=f"k0i{i}")
                for i in range(DT)
            ]
            for i in range(DT):
                nc.gpsimd.dma_start(
                    out=wq_tiles[i], in_=wq_p.ap()[i * P : (i + 1) * P, :]
                )
                nc.sync.dma_start(
                    out=q0_tiles[i], in_=q_ps[0].ap()[i * P : (i + 1) * P, :]
                )
            for i in range(DT):
                nc.gpsimd.dma_start(
                    out=wk_tiles[i], in_=wk_p.ap()[i * P : (i + 1) * P, :]
                )
                nc.sync.dma_start(
                    out=k0_tiles[i], in_=k_ps[0].ap()[i * P : (i + 1) * P, :]
                )
            qins = {0: q0_tiles}
            kins = {0: k0_tiles}
            for s in range(1, S):
                qt = in_pool.tile([P, DT, NQs[s]], F16, tag=f"qin{s}")
                nc.sync.dma_start(
                    out=qt, in_=q_ps[s].ap().rearrange("(dt p) l -> p dt l", p=P)
                )
                kt = in_pool.tile([P, DT, NKs[s]], F16, tag=f"kin{s}")
                nc.sync.dma_start(
                    out=kt, in_=k_ps[s].ap().rearrange("(dt p) l -> p dt l", p=P)
                )
                qins[s] = qt
                kins[s] = kt

            qact = act_pool.tile([P, HT, NQmax], F16, tag="qact")
            kact = act_pool.tile([P, HT, NKmax], F16, tag="kact")

            # ---- PE p-state warm-up: the PE clock ramps 0.65 -> 2.4 GHz
            # only after sustained activity.  Burn dummy matmuls on a
            # memset tile while the first weight/input DMAs are in flight
            # so the real stream starts at full clock.
            warm = const_pool.tile([P, 512], F16, tag="warm")
            nc.gpsimd.memset(warm, 0.0)
            wps = ps_pool.tile([P, 2, 512], F32, tag="ps", name="warmps")
            for i in range(6):
                nc.tensor.matmul(
                    wps[:, 0, 0:512],
                    lhsT=warm[:, 0:P],
                    rhs=warm[:, 0:512],
                    start=True,
                    stop=True,
                )

            for s in range(S):
                NQ, NK = NQs[s], NKs[s]
                cq, ck = _chunks(NQ), _chunks(NK)

                for (wtiles, bsb, dst, N, cc, src) in (
                    (wq_tiles, bq_sb, qact, NQ, cq, "q"),
                    (wk_tiles, bk_sb, kact, NK, ck, "k"),
                ):
                    if s == 0:
                        ins = qins[0] if src == "q" else kins[0]
                        # DMA-paced: 4 concurrent ht accumulations consume
                        # each arriving slice; 4 tiles x 2 banks = all PSUM.
                        for hg in (0, 4):
                            pss = [
                                ps_pool.tile(
                                    [P, 2, 512], F32, tag="ps",
                                    name=f"ps{src}{hg}_{i}",
                                )
                                for i in range(4)
                            ]
                            for dt_i in range(DT):
                                for i in range(4):
                                    for ci, (c0, cw) in enumerate(cc):
                                        nc.tensor.matmul(
                                            pss[i][:, ci, 0:cw],
                                            lhsT=wtiles[dt_i][
                                                :, (hg + i) * P : (hg + i + 1) * P
                                            ],
                                            rhs=ins[dt_i][:, c0 : c0 + cw],
                                            start=(dt_i == 0),
                                            stop=(dt_i == DT - 1),
                                        )
                            for i in range(4):
                                _relu(
                                    nc,
                                    dst[:, hg + i, 0:N],
                                    pss[i],
                                    cc,
                                    bsb[:, hg + i : hg + i + 1],
                                )
                    else:
                        ins = qins[s] if src == "q" else kins[s]
                        for ht in range(HT):
                            ps = ps_pool.tile([P, 2, 512], F32, tag="ps")
                            for dt_i in range(DT):
                                for ci, (c0, cw) in enumerate(cc):
                                    nc.tensor.matmul(
                                        ps[:, ci, 0:cw],
                                        lhsT=wtiles[dt_i][:, ht * P : (ht + 1) * P],
                                        rhs=ins[:, dt_i, c0 : c0 + cw],
                                        start=(dt_i == 0),
                                        stop=(dt_i == DT - 1),
                                    )
                            _relu(
                                nc, dst[:, ht, 0:N], ps, cc, bsb[:, ht : ht + 1]
                            )

                # ---- logits + softmax per 128-row query tile ----
                for r0 in range(0, NQ, P):
                    rw = min(P, NQ - r0)
                    ps2 = ps_pool.tile([P, 2, 512], F32, tag="ps")
                    for ht in range(HT):
                        for ci, (c0, cw) in enumerate(ck):
                            nc.tensor.matmul(
                                ps2[0:rw, ci, 0:cw],
                                lhsT=qact[:, ht, r0 : r0 + rw],
                                rhs=kact[:, ht, c0 : c0 + cw],
                                start=(ht == 0),
                                stop=(ht == HT - 1),
                            )
                    negmx = stat_pool.tile([P, 1], F32, tag="negmx")
                    w_sb = wout_pool.tile([P, NKmax], F16, tag="w")
                    ssum = stat_pool.tile([P, 1], F32, tag="ssum")
                    if len(ck) == 1:
                        nc.vector.reduce_max(
                            out=negmx[0:rw],
                            in_=ps2[0:rw, 0, 0:NK],
                            axis=mybir.AxisListType.X,
                            negate=True,
                        )
                        nc.scalar.activation(
                            out=w_sb[0:rw, 0:NK],
                            in_=ps2[0:rw, 0, 0:NK],
                            func=mybir.ActivationFunctionType.Exp,
                            bias=negmx[0:rw],
                            scale=1.0,
                            accum_out=ssum[0:rw],
                        )
                    else:
                        cw = ck[0][1]
                        nc.vector.reduce_max(
                            out=negmx[0:rw],
                            in_=ps2[0:rw, :, 0:cw],
                            axis=mybir.AxisListType.XY,
                            negate=True,
                        )
                        nc.scalar.activation(
                            out=w_sb[0:rw, 0:NK].rearrange("p (a b) -> p a b", a=2),
                            in_=ps2[0:rw, :, 0:cw],
                            func=mybir.ActivationFunctionType.Exp,
                            bias=negmx[0:rw],
                            scale=1.0,
                            accum_out=ssum[0:rw],
                        )
                    rq = stat_pool.tile([P, 1], F32, tag="rq")
                    nc.vector.reciprocal(out=rq[0:rw], in_=ssum[0:rw])
                    nc.vector.tensor_scalar_mul(
                        out=w_sb[0:rw, 0:NK], in0=w_sb[0:rw, 0:NK], scalar1=rq[0:rw]
                    )
                    nc.gpsimd.dma_start(
                        out=out_ps[s].ap()[r0 : r0 + rw, :], in_=w_sb[0:rw, 0:NK]
                    )

    if split:
        split_multiwaits(nc)
    return nc


def _round8(n):
    return max(8, (n + 7) // 8 * 8)


def _slot_cost(nq, nk):
    # streamed PE rows: two projections (HT*DT matmul groups each) plus
    # logits (ceil(nq/128) row tiles x HT accumulation steps)
    nt = (nq + P - 1) // P
    return DT * HT * (nq + nk) + nt * HT * nk


def _assign_slots(qc, kc):
    """Partition the 32 batches into 4 slots of 8 (one batch per core per
    slot) minimizing total streamed matmul rows.  Greedy + hill climb,
    deterministic."""
    import random

    nb = len(qc)
    order = sorted(
        range(nb), key=lambda b: -_slot_cost(qc[b], kc[b])
    )
    slots = [order[s * N_CORES : (s + 1) * N_CORES] for s in range(B_LOC)]

    def total(sl):
        t = 0
        for idxs in sl:
            nq = _round8(max(qc[b] for b in idxs))
            nk = _round8(max(kc[b] for b in idxs))
            t += _slot_cost(nq, nk)
        return t

    best = total(slots)
    rng = random.Random(1)
    for _ in range(20000):
        s1, s2 = rng.randrange(B_LOC), rng.randrange(B_LOC)
        if s1 == s2:
            continue
        i, j = rng.randrange(N_CORES), rng.randrange(N_CORES)
        slots[s1][i], slots[s2][j] = slots[s2][j], slots[s1][i]
        t = total(slots)
        if t <= best:
            best = t
        else:
            slots[s1][i], slots[s2][j] = slots[s2][j], slots[s1][i]

    # order slots: smallest first (shortest DMA prefix), largest second
    # (fully overlapped), trailing slots shrinking (shorter tail)
    sized = []
    for idxs in slots:
        nq = _round8(max(qc[b] for b in idxs))
        nk = _round8(max(kc[b] for b in idxs))
        sized.append((nq, nk, idxs))
    sized.sort(key=lambda t: t[0] + t[1])
    sized = [sized[0], sized[3], sized[2], sized[1]]
    NQs = [t[0] for t in sized]
    NKs = [t[1] for t in sized]
    slot_batches = [t[2] for t in sized]
    return NQs, NKs, slot_batches


_PROG_CACHE = {}


def _get_prog(NQs, NKs):
    key = (tuple(NQs), tuple(NKs))
    if key not in _PROG_CACHE:
        _PROG_CACHE[key] = build_bass_slotted(NQs, NKs)
    return _PROG_CACHE[key]


def _run(nc, in_maps):
    global LAST_RESULT
    kw = {}
    if TRACE_TMPDIR is not None:
        kw = dict(trace=True, tmpdir=TRACE_TMPDIR)
    res = run_bass_kernel_spmd(nc, in_maps, list(range(N_CORES)), **kw)
    LAST_RESULT = res
    return res


def kernel_packed(query, key, query_mask, key_mask, Wq, bq, Wk, bk):
    qc = np.count_nonzero(query_mask, axis=1)
    kc = np.count_nonzero(key_mask, axis=1)
    NQs, NKs, slot_batches = _assign_slots(qc, kc)
    nc = _get_prog(NQs, NKs)

    WqT = np.ascontiguousarray(Wq.T).astype(np.float16)
    WkT = np.ascontiguousarray(Wk.T).astype(np.float16)
    bq32 = np.ascontiguousarray(bq, dtype=np.float32)
    bk32 = np.ascontiguousarray(bk, dtype=np.float32)

    qidx = {}
    kidx = {}
    in_maps = [
        {"WqT": WqT, "WkT": WkT, "bq": bq32, "bk": bk32} for _ in range(N_CORES)
    ]
    for s in range(B_LOC):
        NQ, NK = NQs[s], NKs[s]
        for c in range(N_CORES):
            b = slot_batches[s][c]
            qi = np.nonzero(query_mask[b])[0]
            ki = np.nonzero(key_mask[b])[0]
            qT = np.zeros((D, NQ), np.float16)
            kT = np.zeros((D, NK), np.float16)
            qT[:, : len(qi)] = query[b][qi].T.astype(np.float16)
            kT[:, : len(ki)] = key[b][ki].T.astype(np.float16)
            in_maps[c][f"qT{s}"] = qT
            in_maps[c][f"kT{s}"] = kT
            qidx[b] = qi
            kidx[b] = ki

    res = _run(nc, in_maps)

    out = np.zeros((B, L, L), np.float32)
    for s in range(B_LOC):
        for c in range(N_CORES):
            b = slot_batches[s][c]
            qi, ki = qidx[b], kidx[b]
            packed = res.results[c][f"out{s}"]
            out[b][np.ix_(qi, ki)] = packed[: len(qi), : len(ki)].astype(np.float32)
    return out


# ---------------------------------------------------------------------------
# Dense fallback (fp32r, key-mask as additive bias) — only used when the
# packed path's assumptions do not hold (bk != 0 or an all-masked key row).
# ---------------------------------------------------------------------------
CH = 512
LT = L // P
NCH = L // CH


def build_bass_dense(b_loc=B_LOC, split=True, mmdt=F32R):
    att_dt = mmdt
    nc = bass.Bass()
    qT_p = nc.declare_dram_parameter("qT", [b_loc, D, L], mmdt, isOutput=False)
    kT_p = nc.declare_dram_parameter("kT", [b_loc, D, L], mmdt, isOutput=False)
    wq_p = nc.declare_dram_parameter("WqT", [D, HID], mmdt, isOutput=False)
    wk_p = nc.declare_dram_parameter("WkT", [D, HID], mmdt, isOutput=False)
    bq_p = nc.declare_dram_parameter("bq", [HID], F32, isOutput=False)
    bk_p = nc.declare_dram_parameter("bk", [HID], F32, isOutput=False)
    nb_p = nc.declare_dram_parameter("negbias", [b_loc, L], F32, isOutput=False)
    qm_p = nc.declare_dram_parameter("qmask", [b_loc, L], F32, isOutput=False)
    out_p = nc.declare_dram_parameter("out", [b_loc, L, L], F32, isOutput=True)

    qT = qT_p.ap()
    kT = kT_p.ap()
    out = out_p.ap()

    with tile.TileContext(nc) as tc:
        with (
            tc.tile_pool(name="wsb", bufs=1) as w_pool,
            tc.tile_pool(name="const", bufs=1) as const_pool,
            tc.tile_pool(name="inp", bufs=2) as in_pool,
            tc.tile_pool(name="act", bufs=1) as act_pool,
            tc.tile_pool(name="nb", bufs=2) as nb_pool,
            tc.tile_pool(name="msk", bufs=2) as msk_pool,
            tc.tile_pool(name="wout", bufs=2) as wout_pool,
            tc.tile_pool(name="stat", bufs=4) as stat_pool,
            tc.tile_pool(name="psA", bufs=2, space="PSUM") as psA,
            tc.tile_pool(name="psB", bufs=3, space="PSUM") as psB,
        ):
            wq_sb = w_pool.tile([P, DT, HID], mmdt, tag="wq")
            nc.sync.dma_start(
                out=wq_sb, in_=wq_p.ap().rearrange("(dt p) h -> p dt h", p=P)
            )
            wk_sb = w_pool.tile([P, DT, HID], mmdt, tag="wk")
            nc.sync.dma_start(
                out=wk_sb, in_=wk_p.ap().rearrange("(dt p) h -> p dt h", p=P)
            )
            bq_sb = const_pool.tile([P, HT], F32, tag="bq")
            nc.sync.dma_start(out=bq_sb, in_=bq_p.ap().rearrange("(t p) -> p t", p=P))
            bk_sb = const_pool.tile([P, HT], F32, tag="bk")
            nc.sync.dma_start(out=bk_sb, in_=bk_p.ap().rearrange("(t p) -> p t", p=P))
            qm_sb = const_pool.tile([P, b_loc, LT], F32, tag="qm")
            nc.sync.dma_start(
                out=qm_sb, in_=qm_p.ap().rearrange("b (t p) -> p b t", p=P)
            )

            for b in range(b_loc):
                nb_sb = nb_pool.tile([P, L], F32, tag="nb")
                nb_row = nb_p.ap()[b]
                nb_bcast = bass.AP(
                    tensor=nb_row.tensor,
                    offset=nb_row.offset,
                    ap=[[0, P], nb_row.ap[0]],
                )
                nc.sync.dma_start(out=nb_sb, in_=nb_bcast)

                qT_act = act_pool.tile([P, HT, L], att_dt, tag="qact")
                kT_act = act_pool.tile([P, HT, L], att_dt, tag="kact")
                for src, wsb, bsb, dst in (
                    (qT, wq_sb, bq_sb, qT_act),
                    (kT, wk_sb, bk_sb, kT_act),
                ):
                    for lc in range(NCH):
                        it = in_pool.tile([P, DT, CH], mmdt, tag="in")
                        nc.sync.dma_start(
                            out=it,
                            in_=src[b, :, lc * CH : (lc + 1) * CH].rearrange(
                                "(dt p) l -> p dt l", p=P
                            ),
                        )
                        for ht in range(HT):
                            ps = psA.tile([P, CH], F32, tag="psA")
                            for dt_i in range(DT):
                                nc.tensor.matmul(
                                    ps,
                                    lhsT=wsb[:, dt_i, ht * P : (ht + 1) * P],
                                    rhs=it[:, dt_i, :],
                                    start=(dt_i == 0),
                                    stop=(dt_i == DT - 1),
                                )
                            nc.scalar.activation(
                                out=dst[:, ht, lc * CH : (lc + 1) * CH],
                                in_=ps,
                                func=mybir.ActivationFunctionType.Relu,
                                bias=bsb[:, ht : ht + 1],
                                scale=1.0,
                            )

                for j in range(LT):
                    ps2 = psB.tile([P, L], F32, tag="psB")
                    for c in range(NCH):
                        for ht in range(HT):
                            nc.tensor.matmul(
                                ps2[:, c * CH : (c + 1) * CH],
                                lhsT=qT_act[:, ht, j * P : (j + 1) * P],
                                rhs=kT_act[:, ht, c * CH : (c + 1) * CH],
                                start=(ht == 0),
                                stop=(ht == HT - 1),
                            )
                    masked = msk_pool.tile([P, L], F32, tag="msk")
                    nc.vector.tensor_add(out=masked, in0=ps2, in1=nb_sb)
                    negmx = stat_pool.tile([P, 1], F32, tag="negmx")
                    nc.vector.reduce_max(
                        out=negmx, in_=masked, axis=mybir.AxisListType.X, negate=True
                    )
                    w_sb = wout_pool.tile([P, L], F32, tag="w")
                    ssum = stat_pool.tile([P, 1], F32, tag="ssum")
                    nc.scalar.activation(
                        out=w_sb,
                        in_=masked,
                        func=mybir.ActivationFunctionType.Exp,
                        bias=negmx,
                        scale=1.0,
                        accum_out=ssum,
                    )
                    rq = stat_pool.tile([P, 1], F32, tag="rq")
                    nc.vector.reciprocal(out=rq, in_=ssum)
                    nc.vector.tensor_mul(out=rq, in0=rq, in1=qm_sb[:, b, j : j + 1])
                    nc.vector.tensor_scalar_mul(out=w_sb, in0=w_sb, scalar1=rq)
                    nc.sync.dma_start(out=out[b, j * P : (j + 1) * P, :], in_=w_sb)

    if split:
        split_multiwaits(nc)
    return nc


def kernel_dense(query, key, query_mask, key_mask, Wq, bq, Wk, bk):
    nc = build_bass_dense()
    qT = np.ascontiguousarray(np.transpose(query, (0, 2, 1)), dtype=np.float32)
    kT = np.ascontiguousarray(np.transpose(key, (0, 2, 1)), dtype=np.float32)
    WqT = np.ascontiguousarray(Wq.T, dtype=np.float32)
    WkT = np.ascontiguousarray(Wk.T, dtype=np.float32)
    bq = np.ascontiguousarray(bq, dtype=np.float32)
    bk = np.ascontiguousarray(bk, dtype=np.float32)
    negbias = (key_mask.astype(np.float32) - 1.0) * (-NEG)
    qmaskf = query_mask.astype(np.float32)
    in_maps = []
    for c in range(N_CORES):
        s = slice(c * B_LOC, (c + 1) * B_LOC)
        in_maps.append(
            {
                "qT": qT[s],
                "kT": kT[s],
                "WqT": WqT,
                "WkT": WkT,
                "bq": bq,
                "bk": bk,
                "negbias": negbias[s],
                "qmask": qmaskf[s],
            }
        )
    res = _run(nc, in_maps)
    return np.concatenate(
        [res.results[c]["out"] for c in range(N_CORES)], axis=0
    ).astype(np.float32)


def kernel(**inputs):
    query = np.asarray(inputs["query"], dtype=np.float32)
    key = np.asarray(inputs["key"], dtype=np.float32)
    query_mask = np.asarray(inputs["query_mask"])
    key_mask = np.asarray(inputs["key_mask"])
    Wq = np.asarray(inputs["Wq"], dtype=np.float32)
    bq = np.asarray(inputs["bq"], dtype=np.float32)
    Wk = np.asarray(inputs["Wk"], dtype=np.float32)
    bk = np.asarray(inputs["bk"], dtype=np.float32)

    kc = np.count_nonzero(key_mask, axis=1)
    packed_ok = bool(np.all(bk == 0.0)) and int(kc.min()) > 0
    if packed_ok:
        return kernel_packed(query, key, query_mask, key_mask, Wq, bq, Wk, bk)
    return kernel_dense(query, key, query_mask, key_mask, Wq, bq, Wk, bk)


# revision 9
# speedup vs baseline: 1.0557x; 1.0557x over previous
"""Masked attention-weights kernel for Trainium2, 8-core data-parallel.

Computes, per batch b:
    q = relu(query @ Wq.T + bq)          [B, LQ, HID]
    k = relu(key   @ Wk.T + bk)          [B, LK, HID]
    logits = q @ k.T                     [B, LQ, LK]
    w = softmax(where(key_mask, logits, -1e9), axis=-1) * query_mask[:, :, None]

Strategy (fast path):
  * Data-parallel over batch B=32 across 8 NeuronCores, 4 batches ("slots")
    per core.  Batches are reassigned to (core, slot) so that per-slot
    packed sizes are minimized jointly.
  * Mask packing: only unmasked queries/keys are shipped, padded per slot
    to NQ[s] / NK[s] (multiples of 8).  Padded key columns have zero
    activations (relu(0*W + bk) with bk == 0), so after the row-max
    subtraction their softmax weight is exp(-max) ~ e^-150 -> flushes to
    0; they are additionally discarded by the host scatter, as are padded
    query rows.  No -1e9 bias tensor is needed at all (guarded: falls
    back to the dense kernel if bk != 0 or a key row is fully masked).
  * All matmul operands in fp16 (full PE rate, half the DMA/SBUF of
    fp32r; fp32 PSUM accumulation keeps the contraction exact).
    Measured end-to-end L2 error vs the fp32 reference: ~2.4e-3.
  * Softmax straight out of PSUM: reduce_max (vector) -> exp+accum
    (scalar, bias=-max) -> reciprocal + scale (vector) -> fp16 DMA out
    on the gpsimd queue (so output never queues behind input DMAs).
"""

import numpy as np

import concourse.bass as bass
import concourse.tile as tile
from concourse import mybir
from concourse.bass_utils import run_bass_kernel_spmd

N_CORES = 8
B, L, HID, D = 32, 1024, 1024, 1024
B_LOC = B // N_CORES
P = 128
DT = D // P
HT = HID // P
NEG = -1e9

F32 = mybir.dt.float32
F32R = mybir.dt.float32r
F16 = mybir.dt.float16

# test.py hooks: set TRACE_TMPDIR to profile; LAST_RESULT carries exec_time_ns
TRACE_TMPDIR = None
LAST_RESULT = None


def split_multiwaits(nc):
    """The walrus build in this container supports a single sync-wait per
    instruction; Tile's tail drain (and some scheduled insts) can carry
    several.  Split the extras into wait-only NOPs on the same engine,
    inserted immediately before the original instruction."""
    n_new = 0
    for fn in nc.m.functions:
        for blk in fn.blocks:
            new_insts = []
            for inst in blk.instructions:
                si = inst.sync_info
                if si is not None and si.on_wait is not None and len(si.on_wait) > 1:
                    waits = list(si.on_wait)
                    for w in waits[:-1]:
                        nop = mybir.InstNoOp(
                            name=f"{inst.name}-ws{n_new}", ins=[], outs=[]
                        )
                        nop.engine = inst.engine
                        nop.sync_info = mybir.SyncInfo(on_wait=[w], on_update=[])
                        new_insts.append(nop)
                        n_new += 1
                    si.on_wait = [waits[-1]]
                new_insts.append(inst)
            blk.instructions = new_insts
    return n_new


def _chunks(n):
    """PSUM free-dim chunking: one matmul if it fits a bank, else two equal
    halves (>=256 each keeps full PE rate)."""
    if n <= 512:
        return [(0, n)]
    h = n // 2
    return [(0, h), (h, n - h)]


def _relu(nc, dst2d, ps, chunks, bias_ap):
    if len(chunks) == 1:
        nc.scalar.activation(
            out=dst2d,
            in_=ps[:, 0, 0 : chunks[0][1]],
            func=mybir.ActivationFunctionType.Relu,
            bias=bias_ap,
            scale=1.0,
        )
    else:
        cw = chunks[0][1]
        nc.scalar.activation(
            out=dst2d.rearrange("p (a b) -> p a b", a=2),
            in_=ps[:, :, 0:cw],
            func=mybir.ActivationFunctionType.Relu,
            bias=bias_ap,
            scale=1.0,
        )


def build_bass_slotted(NQs, NKs, split=True):
    """Per-slot packed attention program.  Slot s processes one batch per
    core with packed query width NQs[s] and key width NKs[s]."""
    S = len(NQs)
    NQmax, NKmax = max(NQs), max(NKs)
    nc = bass.Bass()
    wq_p = nc.declare_dram_parameter("WqT", [D, HID], F16, isOutput=False)
    wk_p = nc.declare_dram_parameter("WkT", [D, HID], F16, isOutput=False)
    bq_p = nc.declare_dram_parameter("bq", [HID], F32, isOutput=False)
    bk_p = nc.declare_dram_parameter("bk", [HID], F32, isOutput=False)
    q_ps = [
        nc.declare_dram_parameter(f"qT{s}", [D, NQs[s]], F16, isOutput=False)
        for s in range(S)
    ]
    k_ps = [
        nc.declare_dram_parameter(f"kT{s}", [D, NKs[s]], F16, isOutput=False)
        for s in range(S)
    ]
    out_ps = [
        nc.declare_dram_parameter(f"out{s}", [NQs[s], NKs[s]], F16, isOutput=True)
        for s in range(S)
    ]

    with tile.TileContext(nc) as tc:
        with (
            tc.tile_pool(name="wsb", bufs=1) as w_pool,
            tc.tile_pool(name="const", bufs=1) as const_pool,
            tc.tile_pool(name="inp", bufs=1) as in_pool,
            tc.tile_pool(name="act", bufs=1) as act_pool,
            tc.tile_pool(name="wout", bufs=3) as wout_pool,
            tc.tile_pool(name="stat", bufs=9) as stat_pool,
            tc.tile_pool(name="ps", bufs=4, space="PSUM") as ps_pool,
        ):
            # ---- DMA plan.  Weight slices issue on the gpsimd queue in
            # parallel with slot-0 input slices on the sync queue (the
            # ~650ns/DMA descriptor-generation rate is what paces startup,
            # not bandwidth); biases ride the scalar queue.  Output DMAs
            # also go on the gpsimd queue, so they never wait behind input
            # prefetch on sync.
            # ---- PE p-state warm-up: the PE clock ramps 0.65 -> 2.4 GHz
            # only after sustained activity.  Burn dummy matmuls on a
            # memset tile while the first weight/input DMAs are in flight
            # so the real stream starts at full clock.  The memset is
            # gpsimd's first op so nothing delays it.
            warm = const_pool.tile([P, 512], F16, tag="warm")
            nc.gpsimd.memset(warm, 0.0)
            wps = ps_pool.tile([P, 2, 512], F32, tag="ps", name="warmps")
            for i in range(6):
                nc.tensor.matmul(
                    wps[:, 0, 0:512],
                    lhsT=warm[:, 0:P],
                    rhs=warm[:, 0:512],
                    start=True,
                    stop=True,
                )

            bq_sb = const_pool.tile([P, HT], F32, tag="bq")
            nc.scalar.dma_start(out=bq_sb, in_=bq_p.ap().rearrange("(t p) -> p t", p=P))
            bk_sb = const_pool.tile([P, HT], F32, tag="bk")
            nc.scalar.dma_start(out=bk_sb, in_=bk_p.ap().rearrange("(t p) -> p t", p=P))

            wq_tiles = [
                w_pool.tile([P, HID], F16, tag=f"wq{i}", name=f"wq{i}")
                for i in range(DT)
            ]
            wk_tiles = [
                w_pool.tile([P, HID], F16, tag=f"wk{i}", name=f"wk{i}")
                for i in range(DT)
            ]
            q0_tiles = [
                in_pool.tile([P, NQs[0]], F16, tag=f"q0i{i}", name=f"q0i{i}")
                for i in range(DT)
            ]
            k0_tiles = [
                in_pool.tile([P, NKs[0]], F16, tag=f"k0i{i}", name=f"k0i{i}")
                for i in range(DT)
            ]
            for i in range(DT):
                nc.gpsimd.dma_start(
                    out=wq_tiles[i], in_=wq_p.ap()[i * P : (i + 1) * P, :]
                )
                nc.sync.dma_start(
                    out=q0_tiles[i], in_=q_ps[0].ap()[i * P : (i + 1) * P, :]
                )
            for i in range(DT):
                nc.gpsimd.dma_start(
                    out=wk_tiles[i], in_=wk_p.ap()[i * P : (i + 1) * P, :]
                )
                nc.sync.dma_start(
                    out=k0_tiles[i], in_=k_ps[0].ap()[i * P : (i + 1) * P, :]
                )
            qins = {0: q0_tiles}
            kins = {0: k0_tiles}
            for s in range(1, S):
                qt = in_pool.tile([P, DT, NQs[s]], F16, tag=f"qin{s}")
                nc.sync.dma_start(
                    out=qt, in_=q_ps[s].ap().rearrange("(dt p) l -> p dt l", p=P)
                )
                kt = in_pool.tile([P, DT, NKs[s]], F16, tag=f"kin{s}")
                nc.sync.dma_start(
                    out=kt, in_=k_ps[s].ap().rearrange("(dt p) l -> p dt l", p=P)
                )
                qins[s] = qt
                kins[s] = kt

            qact = act_pool.tile([P, HT, NQmax], F16, tag="qact")
            kact = act_pool.tile([P, HT, NKmax], F16, tag="kact")

            for s in range(S):
                NQ, NK = NQs[s], NKs[s]
                cq, ck = _chunks(NQ), _chunks(NK)

                for (wtiles, bsb, dst, N, cc, src) in (
                    (wq_tiles, bq_sb, qact, NQ, cq, "q"),
                    (wk_tiles, bk_sb, kact, NK, ck, "k"),
                ):
                    if s == 0:
                        ins = qins[0] if src == "q" else kins[0]
                        # DMA-paced: 4 concurrent ht accumulations consume
                        # each arriving slice; 4 tiles x 2 banks = all PSUM.
                        for hg in (0, 4):
                            pss = [
                                ps_pool.tile(
                                    [P, 2, 512], F32, tag="ps",
                                    name=f"ps{src}{hg}_{i}",
                                )
                                for i in range(4)
                            ]
                            for dt_i in range(DT):
                                for i in range(4):
                                    for ci, (c0, cw) in enumerate(cc):
                                        nc.tensor.matmul(
                                            pss[i][:, ci, 0:cw],
                                            lhsT=wtiles[dt_i][
                                                :, (hg + i) * P : (hg + i + 1) * P
                                            ],
                                            rhs=ins[dt_i][:, c0 : c0 + cw],
                                            start=(dt_i == 0),
                                            stop=(dt_i == DT - 1),
                                        )
                            for i in range(4):
                                _relu(
                                    nc,
                                    dst[:, hg + i, 0:N],
                                    pss[i],
                                    cc,
                                    bsb[:, hg + i : hg + i + 1],
                                )
                    else:
                        ins = qins[s] if src == "q" else kins[s]
                        for ht in range(HT):
                            ps = ps_pool.tile([P, 2, 512], F32, tag="ps")
                            for dt_i in range(DT):
                                for ci, (c0, cw) in enumerate(cc):
                                    nc.tensor.matmul(
                                        ps[:, ci, 0:cw],
                                        lhsT=wtiles[dt_i][:, ht * P : (ht + 1) * P],
                                        rhs=ins[:, dt_i, c0 : c0 + cw],
                                        start=(dt_i == 0),
                                        stop=(dt_i == DT - 1),
                                    )
                            _relu(
                                nc, dst[:, ht, 0:N], ps, cc, bsb[:, ht : ht + 1]
                            )

                # ---- logits + softmax per 128-row query tile ----
                for r0 in range(0, NQ, P):
                    rw = min(P, NQ - r0)
                    ps2 = ps_pool.tile([P, 2, 512], F32, tag="ps")
                    for ht in range(HT):
                        for ci, (c0, cw) in enumerate(ck):
                            nc.tensor.matmul(
                                ps2[0:rw, ci, 0:cw],
                                lhsT=qact[:, ht, r0 : r0 + rw],
                                rhs=kact[:, ht, c0 : c0 + cw],
                                start=(ht == 0),
                                stop=(ht == HT - 1),
                            )
                    negmx = stat_pool.tile([P, 1], F32, tag="negmx")
                    w_sb = wout_pool.tile([P, NKmax], F16, tag="w")
                    ssum = stat_pool.tile([P, 1], F32, tag="ssum")
                    if len(ck) == 1:
                        nc.vector.reduce_max(
                            out=negmx[0:rw],
                            in_=ps2[0:rw, 0, 0:NK],
                            axis=mybir.AxisListType.X,
                            negate=True,
                        )
                        nc.scalar.activation(
                            out=w_sb[0:rw, 0:NK],
                            in_=ps2[0:rw, 0, 0:NK],
                            func=mybir.ActivationFunctionType.Exp,
                            bias=negmx[0:rw],
                            scale=1.0,
                            accum_out=ssum[0:rw],
                        )
                    else:
                        cw = ck[0][1]
                        nc.vector.reduce_max(
                            out=negmx[0:rw],
                            in_=ps2[0:rw, :, 0:cw],
                            axis=mybir.AxisListType.XY,
                            negate=True,
                        )
                        nc.scalar.activation(
                            out=w_sb[0:rw, 0:NK].rearrange("p (a b) -> p a b", a=2),
                            in_=ps2[0:rw, :, 0:cw],
                            func=mybir.ActivationFunctionType.Exp,
                            bias=negmx[0:rw],
                            scale=1.0,
                            accum_out=ssum[0:rw],
                        )
                    rq = stat_pool.tile([P, 1], F32, tag="rq")
                    nc.vector.reciprocal(out=rq[0:rw], in_=ssum[0:rw])
                    nc.vector.tensor_scalar_mul(
                        out=w_sb[0:rw, 0:NK], in0=w_sb[0:rw, 0:NK], scalar1=rq[0:rw]
                    )
                    nc.gpsimd.dma_start(
                        out=out_ps[s].ap()[r0 : r0 + rw, :], in_=w_sb[0:rw, 0:NK]
                    )

    if split:
        split_multiwaits(nc)
    return nc


def _round8(n):
    return max(8, (n + 7) // 8 * 8)


def _slot_cost(nq, nk):
    # streamed PE rows: two projections (HT*DT matmul groups each) plus
    # logits (ceil(nq/128) row tiles x HT accumulation steps)
    nt = (nq + P - 1) // P
    return DT * HT * (nq + nk) + nt * HT * nk


def _assign_slots(qc, kc):
    """Partition the 32 batches into 4 slots of 8 (one batch per core per
    slot) minimizing total streamed matmul rows.  Greedy + hill climb,
    deterministic."""
    import random

    nb = len(qc)
    order = sorted(
        range(nb), key=lambda b: -_slot_cost(qc[b], kc[b])
    )
    slots = [order[s * N_CORES : (s + 1) * N_CORES] for s in range(B_LOC)]

    def total(sl):
        t = 0
        for idxs in sl:
            nq = _round8(max(qc[b] for b in idxs))
            nk = _round8(max(kc[b] for b in idxs))
            t += _slot_cost(nq, nk)
        return t

    best = total(slots)
    rng = random.Random(1)
    for _ in range(20000):
        s1, s2 = rng.randrange(B_LOC), rng.randrange(B_LOC)
        if s1 == s2:
            continue
        i, j = rng.randrange(N_CORES), rng.randrange(N_CORES)
        slots[s1][i], slots[s2][j] = slots[s2][j], slots[s1][i]
        t = total(slots)
        if t <= best:
            best = t
        else:
            slots[s1][i], slots[s2][j] = slots[s2][j], slots[s1][i]

    # order slots: smallest first (shortest DMA prefix), largest second
    # (fully overlapped), trailing slots shrinking (shorter tail)
    sized = []
    for idxs in slots:
        nq = _round8(max(qc[b] for b in idxs))
        nk = _round8(max(kc[b] for b in idxs))
        sized.append((nq, nk, idxs))
    sized.sort(key=lambda t: t[0] + t[1])
    sized = [sized[0], sized[3], sized[2], sized[1]]
    NQs = [t[0] for t in sized]
    NKs = [t[1] for t in sized]
    slot_batches = [t[2] for t in sized]
    return NQs, NKs, slot_batches


_PROG_CACHE = {}


def _get_prog(NQs, NKs):
    key = (tuple(NQs), tuple(NKs))
    if key not in _PROG_CACHE:
        _PROG_CACHE[key] = build_bass_slotted(NQs, NKs)
    return _PROG_CACHE[key]


def _run(nc, in_maps):
    global LAST_RESULT
    kw = {}
    if TRACE_TMPDIR is not None:
        kw = dict(trace=True, tmpdir=TRACE_TMPDIR)
    res = run_bass_kernel_spmd(nc, in_maps, list(range(N_CORES)), **kw)
    LAST_RESULT = res
    return res


def kernel_packed(query, key, query_mask, key_mask, Wq, bq, Wk, bk):
    qc = np.count_nonzero(query_mask, axis=1)
    kc = np.count_nonzero(key_mask, axis=1)
    NQs, NKs, slot_batches = _assign_slots(qc, kc)
    nc = _get_prog(NQs, NKs)

    WqT = np.ascontiguousarray(Wq.T).astype(np.float16)
    WkT = np.ascontiguousarray(Wk.T).astype(np.float16)
    bq32 = np.ascontiguousarray(bq, dtype=np.float32)
    bk32 = np.ascontiguousarray(bk, dtype=np.float32)

    qidx = {}
    kidx = {}
    in_maps = [
        {"WqT": WqT, "WkT": WkT, "bq": bq32, "bk": bk32} for _ in range(N_CORES)
    ]
    for s in range(B_LOC):
        NQ, NK = NQs[s], NKs[s]
        for c in range(N_CORES):
            b = slot_batches[s][c]
            qi = np.nonzero(query_mask[b])[0]
            ki = np.nonzero(key_mask[b])[0]
            qT = np.zeros((D, NQ), np.float16)
            kT = np.zeros((D, NK), np.float16)
            qT[:, : len(qi)] = query[b][qi].T.astype(np.float16)
            kT[:, : len(ki)] = key[b][ki].T.astype(np.float16)
            in_maps[c][f"qT{s}"] = qT
            in_maps[c][f"kT{s}"] = kT
            qidx[b] = qi
            kidx[b] = ki

    res = _run(nc, in_maps)

    out = np.zeros((B, L, L), np.float32)
    for s in range(B_LOC):
        for c in range(N_CORES):
            b = slot_batches[s][c]
            qi, ki = qidx[b], kidx[b]
            packed = res.results[c][f"out{s}"]
            out[b][np.ix_(qi, ki)] = packed[: len(qi), : len(ki)].astype(np.float32)
    return out


# ---------------------------------------------------------------------------
# Dense fallback (fp32r, key-mask as additive bias) — only used when the
# packed path's assumptions do not hold (bk != 0 or an all-masked key row).
# ---------------------------------------------------------------------------
CH = 512
LT = L // P
NCH = L // CH


def build_bass_dense(b_loc=B_LOC, split=True, mmdt=F32R):
    att_dt = mmdt
    nc = bass.Bass()
    qT_p = nc.declare_dram_parameter("qT", [b_loc, D, L], mmdt, isOutput=False)
    kT_p = nc.declare_dram_parameter("kT", [b_loc, D, L], mmdt, isOutput=False)
    wq_p = nc.declare_dram_parameter("WqT", [D, HID], mmdt, isOutput=False)
    wk_p = nc.declare_dram_parameter("WkT", [D, HID], mmdt, isOutput=False)
    bq_p = nc.declare_dram_parameter("bq", [HID], F32, isOutput=False)
    bk_p = nc.declare_dram_parameter("bk", [HID], F32, isOutput=False)
    nb_p = nc.declare_dram_parameter("negbias", [b_loc, L], F32, isOutput=False)
    qm_p = nc.declare_dram_parameter("qmask", [b_loc, L], F32, isOutput=False)
    out_p = nc.declare_dram_parameter("out", [b_loc, L, L], F32, isOutput=True)

    qT = qT_p.ap()
    kT = kT_p.ap()
    out = out_p.ap()

    with tile.TileContext(nc) as tc:
        with (
            tc.tile_pool(name="wsb", bufs=1) as w_pool,
            tc.tile_pool(name="const", bufs=1) as const_pool,
            tc.tile_pool(name="inp", bufs=2) as in_pool,
            tc.tile_pool(name="act", bufs=1) as act_pool,
            tc.tile_pool(name="nb", bufs=2) as nb_pool,
            tc.tile_pool(name="msk", bufs=2) as msk_pool,
            tc.tile_pool(name="wout", bufs=2) as wout_pool,
            tc.tile_pool(name="stat", bufs=4) as stat_pool,
            tc.tile_pool(name="psA", bufs=2, space="PSUM") as psA,
            tc.tile_pool(name="psB", bufs=3, space="PSUM") as psB,
        ):
            wq_sb = w_pool.tile([P, DT, HID], mmdt, tag="wq")
            nc.sync.dma_start(
                out=wq_sb, in_=wq_p.ap().rearrange("(dt p) h -> p dt h", p=P)
            )
            wk_sb = w_pool.tile([P, DT, HID], mmdt, tag="wk")
            nc.sync.dma_start(
                out=wk_sb, in_=wk_p.ap().rearrange("(dt p) h -> p dt h", p=P)
            )
            # ---- PE p-state warm-up: the PE clock ramps 0.65 -> 2.4 GHz
            # only after sustained activity.  Burn dummy matmuls on a
            # memset tile while the first weight/input DMAs are in flight
            # so the real stream starts at full clock.  The memset is
            # gpsimd's first op so nothing delays it.
            warm = const_pool.tile([P, 512], F16, tag="warm")
            nc.gpsimd.memset(warm, 0.0)
            wps = ps_pool.tile([P, 2, 512], F32, tag="ps", name="warmps")
            for i in range(6):
                nc.tensor.matmul(
                    wps[:, 0, 0:512],
                    lhsT=warm[:, 0:P],
                    rhs=warm[:, 0:512],
                    start=True,
                    stop=True,
                )

            bq_sb = const_pool.tile([P, HT], F32, tag="bq")
            nc.sync.dma_start(out=bq_sb, in_=bq_p.ap().rearrange("(t p) -> p t", p=P))
            bk_sb = const_pool.tile([P, HT], F32, tag="bk")
            nc.sync.dma_start(out=bk_sb, in_=bk_p.ap().rearrange("(t p) -> p t", p=P))
            qm_sb = const_pool.tile([P, b_loc, LT], F32, tag="qm")
            nc.sync.dma_start(
                out=qm_sb, in_=qm_p.ap().rearrange("b (t p) -> p b t", p=P)
            )

            for b in range(b_loc):
                nb_sb = nb_pool.tile([P, L], F32, tag="nb")
                nb_row = nb_p.ap()[b]
                nb_bcast = bass.AP(
                    tensor=nb_row.tensor,
                    offset=nb_row.offset,
                    ap=[[0, P], nb_row.ap[0]],
                )
                nc.sync.dma_start(out=nb_sb, in_=nb_bcast)

                qT_act = act_pool.tile([P, HT, L], att_dt, tag="qact")
                kT_act = act_pool.tile([P, HT, L], att_dt, tag="kact")
                for src, wsb, bsb, dst in (
                    (qT, wq_sb, bq_sb, qT_act),
                    (kT, wk_sb, bk_sb, kT_act),
                ):
                    for lc in range(NCH):
                        it = in_pool.tile([P, DT, CH], mmdt, tag="in")
                        nc.sync.dma_start(
                            out=it,
                            in_=src[b, :, lc * CH : (lc + 1) * CH].rearrange(
                                "(dt p) l -> p dt l", p=P
                            ),
                        )
                        for ht in range(HT):
                            ps = psA.tile([P, CH], F32, tag="psA")
                            for dt_i in range(DT):
                                nc.tensor.matmul(
                                    ps,
                                    lhsT=wsb[:, dt_i, ht * P : (ht + 1) * P],
                                    rhs=it[:, dt_i, :],
                                    start=(dt_i == 0),
                                    stop=(dt_i == DT - 1),
                                )
                            nc.scalar.activation(
                                out=dst[:, ht, lc * CH : (lc + 1) * CH],
                                in_=ps,
                                func=mybir.ActivationFunctionType.Relu,
                                bias=bsb[:, ht : ht + 1],
                                scale=1.0,
                            )

                for j in range(LT):
                    ps2 = psB.tile([P, L], F32, tag="psB")
                    for c in range(NCH):
                        for ht in range(HT):
                            nc.tensor.matmul(
                                ps2[:, c * CH : (c + 1) * CH],
                                lhsT=qT_act[:, ht, j * P : (j + 1) * P],
                                rhs=kT_act[:, ht, c * CH : (c + 1) * CH],
                                start=(ht == 0),
                                stop=(ht == HT - 1),
                            )
                    masked = msk_pool.tile([P, L], F32, tag="msk")
                    nc.vector.tensor_add(out=masked, in0=ps2, in1=nb_sb)
                    negmx = stat_pool.tile([P, 1], F32, tag="negmx")
                    nc.vector.reduce_max(
                        out=negmx, in_=masked, axis=mybir.AxisListType.X, negate=True
                    )
                    w_sb = wout_pool.tile([P, L], F32, tag="w")
                    ssum = stat_pool.tile([P, 1], F32, tag="ssum")
                    nc.scalar.activation(
                        out=w_sb,
                        in_=masked,
                        func=mybir.ActivationFunctionType.Exp,
                        bias=negmx,
                        scale=1.0,
                        accum_out=ssum,
                    )
                    rq = stat_pool.tile([P, 1], F32, tag="rq")
                    nc.vector.reciprocal(out=rq, in_=ssum)
                    nc.vector.tensor_mul(out=rq, in0=rq, in1=qm_sb[:, b, j : j + 1])
                    nc.vector.tensor_scalar_mul(out=w_sb, in0=w_sb, scalar1=rq)
                    nc.sync.dma_start(out=out[b, j * P : (j + 1) * P, :], in_=w_sb)

    if split:
        split_multiwaits(nc)
    return nc


def kernel_dense(query, key, query_mask, key_mask, Wq, bq, Wk, bk):
    nc = build_bass_dense()
    qT = np.ascontiguousarray(np.transpose(query, (0, 2, 1)), dtype=np.float32)
    kT = np.ascontiguousarray(np.transpose(key, (0, 2, 1)), dtype=np.float32)
    WqT = np.ascontiguousarray(Wq.T, dtype=np.float32)
    WkT = np.ascontiguousarray(Wk.T, dtype=np.float32)
    bq = np.ascontiguousarray(bq, dtype=np.float32)
    bk = np.ascontiguousarray(bk, dtype=np.float32)
    negbias = (key_mask.astype(np.float32) - 1.0) * (-NEG)
    qmaskf = query_mask.astype(np.float32)
    in_maps = []
    for c in range(N_CORES):
        s = slice(c * B_LOC, (c + 1) * B_LOC)
        in_maps.append(
            {
                "qT": qT[s],
                "kT": kT[s],
                "WqT": WqT,
                "WkT": WkT,
                "bq": bq,
                "bk": bk,
                "negbias": negbias[s],
                "qmask": qmaskf[s],
            }
        )
    res = _run(nc, in_maps)
    return np.concatenate(
        [res.results[c]["out"] for c in range(N_CORES)], axis=0
    ).astype(np.float32)


def kernel(**inputs):
    query = np.asarray(inputs["query"], dtype=np.float32)
    key = np.asarray(inputs["key"], dtype=np.float32)
    query_mask = np.asarray(inputs["query_mask"])
    key_mask = np.asarray(inputs["key_mask"])
    Wq = np.asarray(inputs["Wq"], dtype=np.float32)
    bq = np.asarray(inputs["bq"], dtype=np.float32)
    Wk = np.asarray(inputs["Wk"], dtype=np.float32)
    bk = np.asarray(inputs["bk"], dtype=np.float32)

    kc = np.count_nonzero(key_mask, axis=1)
    packed_ok = bool(np.all(bk == 0.0)) and int(kc.min()) > 0
    if packed_ok:
        return kernel_packed(query, key, query_mask, key_mask, Wq, bq, Wk, bk)
    return kernel_dense(query, key, query_mask, key_mask, Wq, bq, Wk, bk)
